# revision 1
# baseline (speedup 1.0000x reference)
"""MoE top-2 routing kernel for 8 Trainium2 NeuronCores.

Strategy (expert-parallel dispatch):
  - Router (logits/softmax/top-2/aux-loss) computed host-side in fp32 numpy;
    the routing decision IS the sharding decision, so it has to precede the
    device dispatch anyway. Margins between 2nd/3rd expert are >=1.7e-5 for
    this input scale, far above fp32 noise, so host routing matches the
    reference's routing deterministically.
  - Tokens are gathered per expert (counts ~2048 each), padded to a fixed
    capacity, and each of the 8 cores runs Y_e = X_e @ W[e] as a dense
    [cap,1024]x[1024,1024] matmul via the production tile_matmul kernel.
  - Host combines: out[n] = sum_k top_w[n,k] * (Y[e_k, pos] + b[e_k]).

Env knobs (for benchmarking from test.py; defaults are the shipped config):
  BASS_MOE_DTYPE  = float32 | bfloat16 | float32r   (matmul input dtype)
  BASS_MOE_TRACE  = 1 to run with NTFF tracing (needs antenv.axon_hooks shim)
"""

import os

import numpy as np
import ml_dtypes

N_TOKENS = 8192
D_IN = 1024
D_OUT = 1024
N_EXPERTS = 8
TOP_K = 2
LB_WEIGHT = 0.01

CAP_MIN = 2176  # >= max expert load for the reference input (2121), mult of 128

MM_DTYPE = os.environ.get("BASS_MOE_DTYPE", "float32")

_prog_cache = {}
last_results = None  # BassKernelResults of the most recent device run


def _np_dtype(dt_name):
    return {
        "float32": np.float32,
        "float32r": np.float32,
        "bfloat16": ml_dtypes.bfloat16,
    }[dt_name]


def _build_program(cap, dt_name):
    key = (cap, dt_name)
    if key in _prog_cache:
        return _prog_cache[key]

    import concourse.mybir as mybir
    import concourse.tile as tile
    from concourse import bacc
    from concourse.kernels.tile_matmul import matmul_tile_kernel

    mm_dt = getattr(mybir.dt, dt_name)
    nc = bacc.Bacc("TRN2", target_bir_lowering=False, debug=False)
    xT = nc.dram_tensor("xT", [D_IN, cap], mm_dt, kind="ExternalInput").ap()
    w = nc.dram_tensor("w", [D_IN, D_OUT], mm_dt, kind="ExternalInput").ap()
    y = nc.dram_tensor("y", [cap, D_OUT], mybir.dt.float32, kind="ExternalOutput").ap()
    with tile.TileContext(nc) as tc:
        matmul_tile_kernel(tc, xT, w, y)
    nc.compile()
    _prog_cache[key] = nc
    return nc


def _route(x, Wr, br):
    """fp32 router identical to the reference's routing decisions."""
    logits = x @ Wr.T + br  # [N, E] fp32
    m = logits.max(axis=-1, keepdims=True)
    e = np.exp(logits - m)
    probs = e / e.sum(axis=-1, keepdims=True)  # fp32
    # descending stable sort == jax.lax.top_k tie-breaking (lowest index wins)
    top_i = np.argsort(-probs, axis=-1, kind="stable")[:, :TOP_K]
    top_w = np.take_along_axis(probs, top_i, axis=-1)
    top_w = top_w / top_w.sum(axis=-1, keepdims=True)
    mean_prob = probs.mean(axis=0, dtype=np.float64)
    aux = np.float32(np.mean((mean_prob - 1.0 / N_EXPERTS) ** 2) * LB_WEIGHT)
    return probs, top_i, top_w, aux


def kernel(x, W, b, Wr, br):
    global last_results
    x = np.ascontiguousarray(np.asarray(x, dtype=np.float32))
    W = np.asarray(W, dtype=np.float32)
    b = np.asarray(b, dtype=np.float32)
    Wr = np.asarray(Wr, dtype=np.float32)
    br = np.asarray(br, dtype=np.float32)

    probs, top_i, top_w, aux = _route(x, Wr, br)

    # --- dispatch: gather tokens per expert, pad to capacity ---
    np_dt = _np_dtype(MM_DTYPE)
    idx = [np.where((top_i == e).any(axis=1))[0] for e in range(N_EXPERTS)]
    counts = np.array([len(i) for i in idx])
    cap = max(CAP_MIN, int(-(-counts.max() // 128) * 128))

    in_maps = []
    for e in range(N_EXPERTS):
        xT_e = np.zeros((D_IN, cap), dtype=np_dt)
        xT_e[:, : counts[e]] = x[idx[e]].T.astype(np_dt)
        in_maps.append({"xT": xT_e, "w": W[e].astype(np_dt)})

    # --- device: Y_e = X_e @ W[e] on core e ---
    from concourse.bass_utils import run_bass_kernel_spmd

    nc = _build_program(cap, MM_DTYPE)
    trace = os.environ.get("BASS_MOE_TRACE", "0") == "1"
    kwargs = {}
    if trace:
        kwargs = dict(trace=True, tmpdir=os.environ.get("BASS_MOE_TRACE_DIR"))
    res = run_bass_kernel_spmd(nc, in_maps, list(range(N_EXPERTS)), **kwargs)
    last_results = res
    Y = np.stack([res.results[i]["y"] for i in range(N_EXPERTS)])  # [E, cap, O] f32

    # --- combine: out[n] = sum_k w_k * (Y[e_k, pos_k] + b[e_k]) ---
    pos = np.zeros((N_EXPERTS, N_TOKENS), dtype=np.int64)
    for e in range(N_EXPERTS):
        pos[e, idx[e]] = np.arange(counts[e])
    tok = np.arange(N_TOKENS)
    out = np.zeros((N_TOKENS, D_OUT), dtype=np.float32)
    for k in range(TOP_K):
        ek = top_i[:, k]
        rows = Y[ek, pos[ek, tok]]  # [N, O]
        out += top_w[:, k : k + 1] * (rows + b[ek])

    return out, aux


# revision 4
# speedup vs baseline: 2.4287x; 2.4287x over previous
"""MoE top-2 routing kernel for 8 Trainium2 NeuronCores.

Strategy (expert-parallel dispatch):
  - Router (logits/softmax/top-2/aux-loss) computed host-side in fp32 numpy;
    the routing decision IS the sharding decision, so it has to precede the
    device dispatch anyway. Margins between 2nd/3rd expert are >=1.7e-5 for
    this input scale, far above fp32 noise, so host routing matches the
    reference's routing deterministically.
  - Tokens are gathered per expert (counts ~2048 each), padded to a fixed
    capacity, and each of the 8 cores runs Y_e = X_e @ W[e] as a dense
    [cap,1024]x[1024,1024] matmul with a custom SBUF-resident Tile kernel.
  - Host combines: out[n] = sum_k top_w[n,k] * (Y[e_k, pos] + b[e_k]).

Env knobs (for benchmarking from test.py; defaults are the shipped config):
  BASS_MOE_DTYPE  = bfloat16 | float32 | float32r  (matmul input dtype)
  BASS_MOE_SPLIT3 = 1  bf16 hi/lo split: 3 bf16 matmuls, ~fp32 accuracy
  BASS_MOE_TRACE  = 1 to run with NTFF tracing (needs antenv.axon_hooks shim)
"""

import os

import numpy as np
import ml_dtypes

N_TOKENS = 8192
D_IN = 1024
D_OUT = 1024
N_EXPERTS = 8
TOP_K = 2
LB_WEIGHT = 0.01

CAP_MIN = 2176  # >= max expert load for the reference input (2121), mult of 128

MM_DTYPE = os.environ.get("BASS_MOE_DTYPE", "bfloat16")
SPLIT3 = os.environ.get("BASS_MOE_SPLIT3", "0") == "1"

_prog_cache = {}
last_results = None  # BassKernelResults of the most recent device run


def _np_dtype(dt_name):
    return {
        "float32": np.float32,
        "float32r": np.float32,
        "bfloat16": ml_dtypes.bfloat16,
    }[dt_name]


def _build_program(cap, dt_name, split3):
    key = (cap, dt_name, split3)
    if key in _prog_cache:
        return _prog_cache[key]

    import concourse.mybir as mybir
    import concourse.tile as tile
    from concourse import bacc
    from concourse.bass import ds, ts

    mm_dt = getattr(mybir.dt, dt_name)
    f32 = mybir.dt.float32
    P = 128
    NF = 512  # matmul free dim / PSUM bank
    KO = D_IN // P  # 8 contraction tiles
    MT = cap // P  # token tiles
    NT = D_OUT // NF  # 2 output column tiles

    nc = bacc.Bacc("TRN2", target_bir_lowering=False, debug=False)
    xT = nc.dram_tensor("xT", [D_IN, cap], mm_dt, kind="ExternalInput").ap()
    w = nc.dram_tensor("w", [D_IN, D_OUT], mm_dt, kind="ExternalInput").ap()
    if split3:
        xTl = nc.dram_tensor("xTl", [D_IN, cap], mm_dt, kind="ExternalInput").ap()
        wl = nc.dram_tensor("wl", [D_IN, D_OUT], mm_dt, kind="ExternalInput").ap()
    y = nc.dram_tensor("y", [cap, D_OUT], f32, kind="ExternalOutput").ap()

    xv = xT.rearrange("(a p) c -> p a c", p=P)  # [P, KO, cap]
    wv = w.rearrange("(a p) n -> p a n", p=P)  # [P, KO, D_OUT]
    if split3:
        xlv = xTl.rearrange("(a p) c -> p a c", p=P)
        wlv = wl.rearrange("(a p) n -> p a n", p=P)
    yv = y.rearrange("(m p) n -> m p n", p=P)  # [MT, P, D_OUT]

    with tile.TileContext(nc) as tc:
        with (
            tc.tile_pool(name="const", bufs=1) as cpool,
            tc.tile_pool(name="psum", bufs=3, space="PSUM") as ppool,
            tc.tile_pool(name="outp", bufs=6) as opool,
        ):
            w_sb = cpool.tile([P, KO, D_OUT], mm_dt, tag="w")
            x_sb = cpool.tile([P, KO, cap], mm_dt, tag="x")
            if split3:
                wl_sb = cpool.tile([P, KO, D_OUT], mm_dt, tag="wl")
                xl_sb = cpool.tile([P, KO, cap], mm_dt, tag="xl")
            # weights first (compute for tile m needs all of W but only x[m])
            for k in range(KO):
                nc.sync.dma_start(out=w_sb[:, k, :], in_=wv[:, k, :])
                if split3:
                    nc.sync.dma_start(out=wl_sb[:, k, :], in_=wlv[:, k, :])
            for m in range(MT):
                nc.sync.dma_start(out=x_sb[:, :, ts(m, P)], in_=xv[:, :, ts(m, P)])
                if split3:
                    nc.sync.dma_start(out=xl_sb[:, :, ts(m, P)], in_=xlv[:, :, ts(m, P)])

            for m in range(MT):
                ps = [
                    ppool.tile([P, NF], f32, tag=f"ps{n}", name=f"ps{n}_{m}")
                    for n in range(NT)
                ]
                for k in range(KO):
                    pairs = [(x_sb, w_sb)]
                    if split3:
                        pairs = [(x_sb, w_sb), (x_sb, wl_sb), (xl_sb, w_sb)]
                    for pi, (xs, ws) in enumerate(pairs):
                        for n in range(NT):
                            nc.tensor.matmul(
                                ps[n][:],
                                lhsT=xs[:, k, ts(m, P)],
                                rhs=ws[:, k, ds(n * NF, NF)],
                                start=(k == 0 and pi == 0),
                                stop=(k == KO - 1 and pi == len(pairs) - 1),
                            )
                for n in range(NT):
                    ot = opool.tile([P, NF], f32, tag=f"ot{n}")
                    if n % 2 == 0:
                        nc.vector.tensor_copy(ot[:], ps[n][:])
                    else:
                        nc.scalar.copy(ot[:], ps[n][:])
                    nc.sync.dma_start(out=yv[m][:, ds(n * NF, NF)], in_=ot[:])
    nc.compile()
    _prog_cache[key] = nc
    return nc


def _route(x, Wr, br):
    """fp32 router identical to the reference's routing decisions."""
    logits = x @ Wr.T + br  # [N, E] fp32
    m = logits.max(axis=-1, keepdims=True)
    e = np.exp(logits - m)
    probs = e / e.sum(axis=-1, keepdims=True)  # fp32
    # descending stable sort == jax.lax.top_k tie-breaking (lowest index wins)
    top_i = np.argsort(-probs, axis=-1, kind="stable")[:, :TOP_K]
    top_w = np.take_along_axis(probs, top_i, axis=-1)
    top_w = top_w / top_w.sum(axis=-1, keepdims=True)
    mean_prob = probs.mean(axis=0, dtype=np.float64)
    aux = np.float32(np.mean((mean_prob - 1.0 / N_EXPERTS) ** 2) * LB_WEIGHT)
    return probs, top_i, top_w, aux


def kernel(x, W, b, Wr, br):
    global last_results
    x = np.ascontiguousarray(np.asarray(x, dtype=np.float32))
    W = np.asarray(W, dtype=np.float32)
    b = np.asarray(b, dtype=np.float32)
    Wr = np.asarray(Wr, dtype=np.float32)
    br = np.asarray(br, dtype=np.float32)

    probs, top_i, top_w, aux = _route(x, Wr, br)

    # --- dispatch: gather tokens per expert, pad to capacity ---
    np_dt = _np_dtype(MM_DTYPE)
    idx = [np.where((top_i == e).any(axis=1))[0] for e in range(N_EXPERTS)]
    counts = np.array([len(i) for i in idx])
    cap = max(CAP_MIN, int(-(-counts.max() // 128) * 128))

    in_maps = []
    for e in range(N_EXPERTS):
        xe = x[idx[e]].T  # [D_IN, count] f32
        xT_e = np.zeros((D_IN, cap), dtype=np_dt)
        xT_e[:, : counts[e]] = xe.astype(np_dt)
        m = {"xT": xT_e, "w": W[e].astype(np_dt)}
        if SPLIT3:
            xT_l = np.zeros((D_IN, cap), dtype=np_dt)
            xT_l[:, : counts[e]] = (xe - xT_e[:, : counts[e]].astype(np.float32)).astype(np_dt)
            m["xTl"] = xT_l
            m["wl"] = (W[e] - m["w"].astype(np.float32)).astype(np_dt)
        in_maps.append(m)

    # --- device: Y_e = X_e @ W[e] on core e ---
    from concourse.bass_utils import run_bass_kernel_spmd

    nc = _build_program(cap, MM_DTYPE, SPLIT3)
    trace = os.environ.get("BASS_MOE_TRACE", "0") == "1"
    kwargs = {}
    if trace:
        kwargs = dict(trace=True, tmpdir=os.environ.get("BASS_MOE_TRACE_DIR"))
    res = run_bass_kernel_spmd(nc, in_maps, list(range(N_EXPERTS)), **kwargs)
    last_results = res
    Y = np.stack([res.results[i]["y"] for i in range(N_EXPERTS)])  # [E, cap, O] f32

    # --- combine: out[n] = sum_k w_k * (Y[e_k, pos_k] + b[e_k]) ---
    pos = np.zeros((N_EXPERTS, N_TOKENS), dtype=np.int64)
    for e in range(N_EXPERTS):
        pos[e, idx[e]] = np.arange(counts[e])
    tok = np.arange(N_TOKENS)
    out = np.zeros((N_TOKENS, D_OUT), dtype=np.float32)
    for k in range(TOP_K):
        ek = top_i[:, k]
        rows = Y[ek, pos[ek, tok]]  # [N, O]
        out += top_w[:, k : k + 1] * (rows + b[ek])

    return out, aux


# revision 8
# speedup vs baseline: 2.7577x; 1.1355x over previous
"""MoE top-2 routing kernel for 8 Trainium2 NeuronCores.

Strategy (expert-parallel dispatch):
  - Router (logits/softmax/top-2/aux-loss) computed host-side in fp32 numpy;
    the routing decision IS the sharding decision, so it has to precede the
    device dispatch anyway. Margins between 2nd/3rd expert are >=1.7e-5 for
    this input scale, far above fp32 noise, so host routing matches the
    reference's routing deterministically.
  - Tokens are gathered per expert (counts ~2048 each), padded to a fixed
    capacity, and each of the 8 cores runs Y_e = X_e @ W[e] as a dense
    [cap,1024]x[1024,1024] matmul with a custom SBUF-resident Tile kernel.
  - Host combines: out[n] = sum_k top_w[n,k] * (Y[e_k, pos] + b[e_k]).

Env knobs (for benchmarking from test.py; defaults are the shipped config):
  BASS_MOE_DTYPE  = bfloat16 | float32 | float32r  (matmul input dtype)
  BASS_MOE_SPLIT3 = 1  bf16 hi/lo split: 3 bf16 matmuls, ~fp32 accuracy
  BASS_MOE_TRACE  = 1 to run with NTFF tracing (needs antenv.axon_hooks shim)
"""

import os

import numpy as np
import ml_dtypes

N_TOKENS = 8192
D_IN = 1024
D_OUT = 1024
N_EXPERTS = 8
TOP_K = 2
LB_WEIGHT = 0.01

CAP_MIN = 2176  # >= max expert load for the reference input (2121), mult of 128

MM_DTYPE = os.environ.get("BASS_MOE_DTYPE", "bfloat16")
SPLIT3 = os.environ.get("BASS_MOE_SPLIT3", "0") == "1"

_prog_cache = {}
last_results = None  # BassKernelResults of the most recent device run


def _np_dtype(dt_name):
    return {
        "float32": np.float32,
        "float32r": np.float32,
        "bfloat16": ml_dtypes.bfloat16,
    }[dt_name]


def _build_program(cap, dt_name, split3):
    key = (cap, dt_name, split3)
    if key in _prog_cache:
        return _prog_cache[key]

    import concourse.mybir as mybir
    import concourse.tile as tile
    from concourse import bacc
    from concourse.bass import ds, ts

    mm_dt = getattr(mybir.dt, dt_name)
    f32 = mybir.dt.float32
    P = 128
    NF = 512  # matmul free dim / PSUM bank
    KO = D_IN // P  # 8 contraction tiles
    MT = cap // P  # token tiles
    NT = D_OUT // NF  # 2 output column tiles

    # Host-side packed layouts (one contiguous run per partition per DMA):
    #   xh[p, m*KO*P + k*P + c] = X[m*P + c, k*P + p]   (lhsT tiles, row-major)
    #   wh[p, k*D_OUT + n]      = W[k*P + p, n]
    #   yh[p, m*D_OUT + n]      = Y[m*P + p, n]
    nc = bacc.Bacc("TRN2", target_bir_lowering=False, debug=False)
    xh = nc.dram_tensor("xh", [P, MT * KO * P], mm_dt, kind="ExternalInput").ap()
    wh = nc.dram_tensor("wh", [P, KO * D_OUT], mm_dt, kind="ExternalInput").ap()
    if split3:
        xhl = nc.dram_tensor("xhl", [P, MT * KO * P], mm_dt, kind="ExternalInput").ap()
        whl = nc.dram_tensor("whl", [P, KO * D_OUT], mm_dt, kind="ExternalInput").ap()
    yh = nc.dram_tensor("yh", [P, MT * D_OUT], f32, kind="ExternalOutput").ap()

    with tile.TileContext(nc) as tc:
        with (
            tc.tile_pool(name="const", bufs=1) as cpool,
            tc.tile_pool(name="psum", bufs=3, space="PSUM") as ppool,
            tc.tile_pool(name="outp", bufs=4) as opool,
        ):
            w_sb = cpool.tile([P, KO * D_OUT], mm_dt, tag="w")
            x_sb = cpool.tile([P, MT, KO * P], mm_dt, tag="x")
            if split3:
                wl_sb = cpool.tile([P, KO * D_OUT], mm_dt, tag="wl")
                xl_sb = cpool.tile([P, MT, KO * P], mm_dt, tag="xl")
            # x[m=0] first (first matmul group), then W per-k (pipelines the
            # first group's k-accumulation), then the remaining x tiles.
            nc.sync.dma_start(out=x_sb[:, 0, :], in_=xh[:, ds(0, KO * P)])
            if split3:
                nc.sync.dma_start(out=xl_sb[:, 0, :], in_=xhl[:, ds(0, KO * P)])
            for k in range(KO):
                nc.sync.dma_start(
                    out=w_sb[:, ds(k * D_OUT, D_OUT)], in_=wh[:, ds(k * D_OUT, D_OUT)]
                )
                if split3:
                    nc.sync.dma_start(
                        out=wl_sb[:, ds(k * D_OUT, D_OUT)],
                        in_=whl[:, ds(k * D_OUT, D_OUT)],
                    )
            for m in range(1, MT):
                nc.sync.dma_start(
                    out=x_sb[:, m, :], in_=xh[:, ds(m * KO * P, KO * P)]
                )
                if split3:
                    nc.sync.dma_start(
                        out=xl_sb[:, m, :], in_=xhl[:, ds(m * KO * P, KO * P)]
                    )

            for m in range(MT):
                ps = [
                    ppool.tile([P, NF], f32, tag=f"ps{n}", name=f"ps{n}_{m}")
                    for n in range(NT)
                ]
                for k in range(KO):
                    pairs = [(x_sb, w_sb)]
                    if split3:
                        pairs = [(x_sb, w_sb), (x_sb, wl_sb), (xl_sb, w_sb)]
                    for pi, (xs, ws) in enumerate(pairs):
                        for n in range(NT):
                            nc.tensor.matmul(
                                ps[n][:],
                                lhsT=xs[:, m, ds(k * P, P)],
                                rhs=ws[:, ds(k * D_OUT + n * NF, NF)],
                                start=(k == 0 and pi == 0),
                                stop=(k == KO - 1 and pi == len(pairs) - 1),
                            )
                ot = opool.tile([P, D_OUT], f32, tag="ot")
                for n in range(NT):
                    if n % 2 == 0:
                        nc.vector.tensor_copy(ot[:, ds(n * NF, NF)], ps[n][:])
                    else:
                        nc.scalar.copy(ot[:, ds(n * NF, NF)], ps[n][:])
                nc.gpsimd.dma_start(out=yh[:, ds(m * D_OUT, D_OUT)], in_=ot[:])
    nc.compile()
    _prog_cache[key] = nc
    return nc


def _route(x, Wr, br):
    """fp32 router identical to the reference's routing decisions."""
    logits = x @ Wr.T + br  # [N, E] fp32
    m = logits.max(axis=-1, keepdims=True)
    e = np.exp(logits - m)
    probs = e / e.sum(axis=-1, keepdims=True)  # fp32
    # descending stable sort == jax.lax.top_k tie-breaking (lowest index wins)
    top_i = np.argsort(-probs, axis=-1, kind="stable")[:, :TOP_K]
    top_w = np.take_along_axis(probs, top_i, axis=-1)
    top_w = top_w / top_w.sum(axis=-1, keepdims=True)
    mean_prob = probs.mean(axis=0, dtype=np.float64)
    aux = np.float32(np.mean((mean_prob - 1.0 / N_EXPERTS) ** 2) * LB_WEIGHT)
    return probs, top_i, top_w, aux


def kernel(x, W, b, Wr, br):
    global last_results
    x = np.ascontiguousarray(np.asarray(x, dtype=np.float32))
    W = np.asarray(W, dtype=np.float32)
    b = np.asarray(b, dtype=np.float32)
    Wr = np.asarray(Wr, dtype=np.float32)
    br = np.asarray(br, dtype=np.float32)

    probs, top_i, top_w, aux = _route(x, Wr, br)

    # --- dispatch: gather tokens per expert, pad to capacity ---
    np_dt = _np_dtype(MM_DTYPE)
    idx = [np.where((top_i == e).any(axis=1))[0] for e in range(N_EXPERTS)]
    counts = np.array([len(i) for i in idx])
    cap = max(CAP_MIN, int(-(-counts.max() // 128) * 128))

    P = 128
    KO = D_IN // P
    MT = cap // P

    def pack_x(xg):  # [cap, D_IN] -> [P, MT*KO*P]: xh[p, (m,k,c)] = xg[mP+c, kP+p]
        return np.ascontiguousarray(
            xg.reshape(MT, P, KO, P).transpose(3, 0, 2, 1).reshape(P, MT * KO * P)
        )

    def pack_w(we):  # [D_IN, D_OUT] -> [P, KO*D_OUT]: wh[p, (k,n)] = we[kP+p, n]
        return np.ascontiguousarray(
            we.reshape(KO, P, D_OUT).transpose(1, 0, 2).reshape(P, KO * D_OUT)
        )

    in_maps = []
    for e in range(N_EXPERTS):
        xg = np.zeros((cap, D_IN), dtype=np.float32)
        xg[: counts[e]] = x[idx[e]]
        m = {"xh": pack_x(xg.astype(np_dt)), "wh": pack_w(W[e].astype(np_dt))}
        if SPLIT3:
            xlo = (xg - xg.astype(np_dt).astype(np.float32)).astype(np_dt)
            wlo = (W[e] - W[e].astype(np_dt).astype(np.float32)).astype(np_dt)
            m["xhl"] = pack_x(xlo)
            m["whl"] = pack_w(wlo)
        in_maps.append(m)

    # --- device: Y_e = X_e @ W[e] on core e ---
    from concourse.bass_utils import run_bass_kernel_spmd

    nc = _build_program(cap, MM_DTYPE, SPLIT3)
    trace = os.environ.get("BASS_MOE_TRACE", "0") == "1"
    kwargs = {}
    if trace:
        kwargs = dict(trace=True, tmpdir=os.environ.get("BASS_MOE_TRACE_DIR"))
    res = run_bass_kernel_spmd(nc, in_maps, list(range(N_EXPERTS)), **kwargs)
    last_results = res
    # yh[p, (m,n)] = Y[m*P+p, n]  ->  Y[cap, D_OUT]
    Y = np.stack(
        [
            res.results[i]["yh"]
            .reshape(P, MT, D_OUT)
            .transpose(1, 0, 2)
            .reshape(cap, D_OUT)
            for i in range(N_EXPERTS)
        ]
    )  # [E, cap, O] f32

    # --- combine: out[n] = sum_k w_k * (Y[e_k, pos_k] + b[e_k]) ---
    pos = np.zeros((N_EXPERTS, N_TOKENS), dtype=np.int64)
    for e in range(N_EXPERTS):
        pos[e, idx[e]] = np.arange(counts[e])
    tok = np.arange(N_TOKENS)
    out = np.zeros((N_TOKENS, D_OUT), dtype=np.float32)
    for k in range(TOP_K):
        ek = top_i[:, k]
        rows = Y[ek, pos[ek, tok]]  # [N, O]
        out += top_w[:, k : k + 1] * (rows + b[ek])

    return out, aux
